# revision 40
# baseline (speedup 1.0000x reference)
"""DistGraphConv on 8 TRN2 NeuronCores.

GraphConv (norm='both'): out = rsqrt(deg_in) * ((A @ (x * rsqrt(deg_out))) @ W) + bias
                             = (A_sym @ x) @ W + bias,
where A_sym[d, s] = sum over edges (s->d) of rsqrt(deg_out[s]) * rsqrt(deg_in[d])
(the right norm commutes with the GEMM since it is a per-row scale).

Strategy (1-D dst partition, SPMD single NEFF on cores 0-7):
  - Nodes are split into 128-wide dst "windows"; window w -> (core, position)
    balanced by edge count; each core owns ~49 positions.
  - x is shipped as bf16 (representation change only).
  - Host prep (graph metadata only): bucket edges by (core, position, src-half),
    sort by src, build idx tables and the normalized adjacency one-hot blocks
    (values = rsqrt(deg_out[src])*rsqrt(deg_in[dst]), pure graph metadata).
    Chunk capacities are max-over-cores so all cores share one instruction
    stream; per-core differences are data only.
  - Device, per position:
      dma_gather     : Xg[e,:] = x_bf16[src_e,:]   (256B rows, HBM->SBUF)
      DMA            : O normalized one-hot stream (bf16)
      PE             : psum1[f,d] += Xg_chunk.T @ O_chunk   (h^T scatter-add)
      ACT            : hsT[f,d] = bf16(psum1)
      PE             : psum2[d,fo] = hsT.T @ W
      DVE            : ot = psum2 + bias (broadcast tile);  DMA out.

Performance notes (measured on trn2 via perfetto):
  - The kernel is bound by SWDGE descriptor emission for the per-edge
    gather: each dma_gather occupies one Q7 core pair (pair = queue_num),
    ~10.5 ns/index; 4 queues run 4 pairs in parallel -> ~100k idx/core
    / 4 pairs ~= 270 us floor.  Gather calls are split in half
    (SPLIT_CALLS=2 -> ~196 calls of ~550 idx) and issued LPT
    (largest-first) so the 4 queue makespans stay balanced and the
    drain tail hangs off the smallest positions.
  - Both GCN norms are folded into the one-hot values on the host, so
    the device path is gather -> matmul -> matmul -> +bias with no
    per-edge vector work.
  - Exact-count mode: idx tables pad with -1 and each call's true
    per-core count is passed via num_idxs_reg (value_load from SBUF,
    no min/max bounds -- bounded value_load emits runtime-assert ALU
    ops that halt the device).  The Q7 trims trailing -1s, skipping
    whole padded chunks (~4% emission + ~6% gather bytes).
  - Generation-1 xg tiles (first NXG tile allocations) pad with idx 0
    and a full count so every lane is written: no pool-priming
    memsets, and later generations can trim onto finite stale data.
"""

import os
import sys
import types

import numpy as np

P = 128
HALF = 32768  # int16 gather-index limit
NXG = int(os.environ.get("KERNEL_NXG", "16"))  # xg pool depth (buffers)
WARMUP = bool(int(os.environ.get("KERNEL_WARMUP", "0")))  # per-queue IRAM warmup
NEG_PAD = bool(int(os.environ.get("KERNEL_NEG_PAD", "1")))
EXACT_CNT = bool(int(os.environ.get("KERNEL_EXACT_CNT", "1")))
SPLIT_CALLS = int(os.environ.get("KERNEL_SPLIT_CALLS", "2"))  # gathers per (pos,half)
TGT_CALL = int(os.environ.get("KERNEL_TGT_CALL", "0"))  # target idxs/call (0=off)
QMODE = os.environ.get("KERNEL_QMODE", "load")  # load | rr

_CACHE: dict = {}


# ----------------------------------------------------------------- ntff shim
def _install_ntff_hook_shim():
    """The agent image's antenv lacks axon_hooks; bass_utils imports it when
    trace=True. Provide the module and register the ctypes NTFF hook."""
    try:
        from antenv.axon_hooks import get_axon_ntff_profile_hook  # noqa: F401
        return
    except ImportError:
        pass
    mod = types.ModuleType("antenv.axon_hooks")
    _hook = [None]
    mod.set_axon_ntff_profile_hook = lambda h: _hook.__setitem__(0, h)
    mod.get_axon_ntff_profile_hook = lambda: _hook[0]
    sys.modules["antenv.axon_hooks"] = mod
    import antenv

    antenv.axon_hooks = mod
    try:
        from trn_agent_boot.trn_boot import _ntff_profile_via_ctypes

        mod.set_axon_ntff_profile_hook(
            _ntff_profile_via_ctypes("/opt/axon/libaxon_pjrt.so")
        )
    except Exception:
        pass


# ----------------------------------------------------------------- host prep
def _prep(x, src, dst, weight, bias):
    import ml_dtypes

    n, f = x.shape
    e = src.shape[0]
    n_win = -(-n // P)
    cores = 8
    wpc = -(-n_win // cores)

    deg_out = np.maximum(np.bincount(src, minlength=n), 1).astype(np.float32)
    deg_in = np.maximum(np.bincount(dst, minlength=n), 1).astype(np.float32)
    w_edge = 1.0 / np.sqrt(deg_out[src] * deg_in[dst])  # normalized A values

    win = (dst >> 7).astype(np.int64)
    wcnt = np.bincount(win, minlength=n_win)

    # window -> (core, pos): sort windows by size desc; group of 8 similar
    # windows per position; within a group assign biggest to least-loaded core.
    worder = np.argsort(-wcnt, kind="stable")
    win_to_core = np.zeros(n_win, np.int64)
    win_to_pos = np.zeros(n_win, np.int64)
    pos_to_win = np.full((cores, wpc), -1, np.int64)
    core_load = np.zeros(cores, np.int64)
    for j in range(wpc):
        grp = worder[j * 8:(j + 1) * 8]
        order_c = np.argsort(core_load, kind="stable")
        for i, w in enumerate(grp):
            c = int(order_c[i])
            win_to_core[w] = c
            win_to_pos[w] = j
            pos_to_win[c, j] = w
            core_load[c] += wcnt[w]

    core = win_to_core[win]
    pos = win_to_pos[win]
    half = (src >= HALF).astype(np.int64)

    gkey = (core * wpc + pos) * 2 + half
    order = np.lexsort((src, gkey))
    src_s = src[order]
    dst_s = dst[order]
    w_s = w_edge[order]
    gkey_s = gkey[order]

    n_groups = cores * wpc * 2
    gcnt = np.bincount(gkey_s, minlength=n_groups)
    gstart = np.zeros(n_groups + 1, np.int64)
    np.cumsum(gcnt, out=gstart[1:])
    cnt = gcnt.reshape(cores, wpc, 2)
    cmax = (-(-cnt // P)).max(axis=0)  # [wpc, 2] chunks per (pos, half)
    slot0 = np.zeros((wpc, 2), np.int64)
    s = 0
    for j in range(wpc):
        for h in range(2):
            slot0[j, h] = s
            s += cmax[j, h]
    n_slots = int(s)

    # per-edge placement
    g_c = gkey_s // (wpc * 2)
    g_rem = gkey_s - g_c * (wpc * 2)
    g_j = g_rem >> 1
    g_h = g_rem & 1
    epos = np.arange(e, dtype=np.int64) - gstart[gkey_s]
    lane = epos & 127
    slot = slot0[g_j, g_h] + (epos >> 7)

    # dense normalized one-hot blocks: O[lane, slot*128+dstl] = w_edge
    o_rep = np.zeros((cores, P, n_slots * P), ml_dtypes.bfloat16)
    o_rep[g_c, lane, slot * P + (dst_s & 127)] = w_s

    # gather batching + call splitting. gather-slot order: per batch:
    # [h0: pos j0..][h1: pos j0..]; onehot slot order stays (j, h)-global.
    BPG = int(os.environ.get("KERNEL_BPG", "1"))
    batches = [list(range(b, min(b + BPG, wpc))) for b in range(0, wpc, BPG)]
    # LPT dispatch order baked into the layout: biggest batch first, so the
    # gather-slot/idx-table order equals issue order and the first batch's
    # idx columns sit at the front (loaded as a separate small tile).
    batches.sort(key=lambda bj: -sum(int(cmax[j, 0] + cmax[j, 1])
                                     for j in bj))
    gslot0 = np.zeros((wpc, 2), np.int64)
    bat_g0 = []  # per batch: (g0_h0, B_h0, g0_h1, B_h1)
    s = 0
    for bj in batches:
        b00 = s
        for j in bj:
            gslot0[j, 0] = s
            s += cmax[j, 0]
        b10 = s
        for j in bj:
            gslot0[j, 1] = s
            s += cmax[j, 1]
        bat_g0.append((b00, b10 - b00, b10, s - b10))
    assert s == n_slots

    # per-call real index counts (exact-count mode: num_idxs_reg from SBUF).
    # Call order mirrors the device loop: per batch, per half, per split.
    # Exact counts require trailing-only padding within each call: BPG == 1.
    # The first NXG (batch, half) tiles are generation 1 of the xg pool:
    # they pad with idx 0 and a FULL count so every lane gets written,
    # which removes the need to prime the pool with memsets (later
    # generations overwrite finite gather data, so trimming is safe).
    exact = EXACT_CNT and NEG_PAD and BPG == 1
    call_list = []  # (j, h, lo, hi, full) chunk ranges
    zero_pad = {}  # (j, h) -> True for generation-1 tiles
    if exact:
        tile_ord = 0
        for bj in batches:
            (j,) = bj
            for h in range(2):
                Bt = int(cmax[j, h])
                if Bt == 0:
                    continue
                tile_ord += 1
                full = tile_ord <= NXG
                zero_pad[(j, h)] = full
                nsp = max(1, min(SPLIT_CALLS, Bt))
                bnds = [Bt * k // nsp for k in range(nsp + 1)]
                for k in range(nsp):
                    lo, hi = bnds[k], bnds[k + 1]
                    if hi > lo:
                        call_list.append((j, h, lo, hi, full))
    n_calls = max(1, len(call_list))
    cnts = np.zeros((cores, 1, n_calls), np.int32)
    for ci, (j, h, lo, hi, full) in enumerate(call_list):
        if full:
            cnts[:, 0, ci] = (hi - lo) * P
        else:
            r = cnt[:, j, h] - lo * P
            cnts[:, 0, ci] = np.clip(r, 0, (hi - lo) * P)

    # idx tables in gather-slot order [cores, 16, idx_cols], pad -1 (trimmed)
    pad_val = -1 if NEG_PAD else 0
    idx_cols = n_slots * 8
    idx_tab = np.full((cores, 16, idx_cols), pad_val, np.int16)
    for c in range(cores):
        for j in range(wpc):
            for h in range(2):
                B = int(cmax[j, h])
                if B == 0:
                    continue
                g = (c * wpc + j) * 2 + h
                i0, i1 = gstart[g], gstart[g + 1]
                pv = 0 if zero_pad.get((j, h)) else pad_val
                buf = np.full(B * P, pv, np.int16)
                buf[: i1 - i0] = (src_s[i0:i1] - h * HALF).astype(np.int16)
                cc = int(gslot0[j, h]) * 8
                idx_tab[c, :, cc:cc + B * 8] = buf.reshape(B * 8, 16).T
    idx_tab_full = np.tile(idx_tab, (1, 8, 1))

    bias_b = np.tile(np.asarray(bias, np.float32)[None, :], (P, 1))
    w_bf = np.asarray(weight, np.float32).astype(ml_dtypes.bfloat16)
    x_bf = np.asarray(x, np.float32).astype(ml_dtypes.bfloat16)

    meta = dict(
        n=n, f=f, e=e, n_win=n_win, wpc=wpc, n_slots=n_slots,
        idx_cols=idx_cols, cmax=cmax, slot0=slot0, gslot0=gslot0,
        batches=batches, bat_g0=bat_g0, pos_to_win=pos_to_win,
        exact=exact, n_calls=n_calls,
    )
    in_maps = []
    for c in range(cores):
        m = {
            "x": x_bf,
            "onehot": o_rep[c],
            "idx": idx_tab_full[c],
            "w_bf": w_bf,
            "bias_b": bias_b,
        }
        if exact:
            m["cnts"] = cnts[c]
        in_maps.append(m)
    return meta, in_maps


# ------------------------------------------------------------- device build
def _build(meta):
    import concourse.bacc as bacc
    import concourse.mybir as mybir
    import concourse.tile as tile
    from concourse.library_config import mlp

    n, f = meta["n"], meta["f"]
    wpc = meta["wpc"]
    n_slots = meta["n_slots"]
    idx_cols = meta["idx_cols"]
    cmax = meta["cmax"]
    slot0 = meta["slot0"]
    gslot0 = meta["gslot0"]
    batches = meta["batches"]
    bat_g0 = meta["bat_g0"]
    fp32 = mybir.dt.float32
    bf16 = mybir.dt.bfloat16

    exact = meta["exact"]
    n_calls = meta["n_calls"]

    nc = bacc.Bacc("TRN2", target_bir_lowering=False, debug=False,
                   num_swdge_queues=4)
    x_d = nc.declare_dram_parameter("x", [n, f], bf16, isOutput=False)
    cnts_d = (nc.declare_dram_parameter("cnts", [1, n_calls], mybir.dt.int32,
                                        isOutput=False) if exact else None)
    oh_d = nc.declare_dram_parameter("onehot", [P, n_slots * P], bf16,
                                     isOutput=False)
    idx_d = nc.declare_dram_parameter("idx", [P, idx_cols], mybir.dt.int16,
                                      isOutput=False)
    w_d = nc.declare_dram_parameter("w_bf", [f, f], bf16, isOutput=False)
    biasb_d = nc.declare_dram_parameter("bias_b", [P, f], fp32, isOutput=False)
    out_d = nc.declare_dram_parameter("out", [wpc * P, f], fp32, isOutput=True)

    x_lo = x_d[0:min(HALF, n), :]
    x_hi = x_d[HALF:n, :] if n > HALF else None

    Bmax = max(max(bg[1], bg[3]) for bg in bat_g0)  # chunks per gather call
    gq = [0, 0, 0, 0]
    rr = [0]

    def next_q(nidx):
        if QMODE == "rr":
            q = rr[0] % 4
            rr[0] += 1
        else:
            q = min(range(4), key=lambda i: gq[i])
        gq[q] += nidx
        return q

    with tile.TileContext(nc) as tc:
        nc.gpsimd.load_library(mlp)
        with (
            tc.tile_pool(name="const", bufs=1) as cpool,
            tc.tile_pool(name="xg", bufs=NXG) as xgpool,
            tc.tile_pool(name="oh", bufs=6) as ohpool,
            tc.tile_pool(name="wout", bufs=6) as wout,
            tc.tile_pool(name="ps1", bufs=6, space="PSUM") as ps1pool,
            tc.tile_pool(name="ps2", bufs=2, space="PSUM") as ps2pool,
        ):
            # one-time loads; counts (tiny, gates every gather) first.
            # idx split into two TILES: Tile deps are tile-granular, so the
            # first batch's gathers only wait on the small idx_a DMA.
            cnts_t = None
            if exact:
                cnts_t = cpool.tile([1, n_calls], mybir.dt.int32)
                nc.sync.dma_start(cnts_t[:], cnts_d[:])
            c0 = int(bat_g0[0][3] + bat_g0[0][2] - bat_g0[0][0]) * 8
            c0 = max(8, min(c0, idx_cols))
            idx_a = cpool.tile([P, c0], mybir.dt.int16)
            nc.sync.dma_start(idx_a[:], idx_d[:, 0:c0])
            idx_b = None
            if idx_cols > c0:
                idx_b = cpool.tile([P, idx_cols - c0], mybir.dt.int16)
                nc.sync.dma_start(idx_b[:], idx_d[:, c0:])
            w_t = cpool.tile([f, f], bf16)
            nc.sync.dma_start(w_t[:], w_d[:])
            biasb_t = cpool.tile([P, f], fp32)
            nc.sync.dma_start(biasb_t[:], biasb_d[:])

            if NEG_PAD and not exact:
                # prime the xg pool buffers so skipped lanes stay finite
                # (exact mode instead writes every lane of generation-1
                # tiles via zero-pad + full counts -- no priming needed)
                for i in range(NXG):
                    t = xgpool.tile([P, Bmax, f], bf16, tag="xg",
                                    name=f"xgz{i}")
                    nc.vector.memset(t[:], 0.0)

            if WARMUP:
                # one tiny gather per queue: pays each Q7 pair's ~6us
                # library-IRAM load while the idx table is still streaming in
                for q in range(4):
                    wut = cpool.tile([P, 1, f], bf16)
                    nc.gpsimd.dma_gather(
                        wut[:, 0:1, :], x_lo,
                        idx_a[:, 0:8],
                        P, P, f, single_packet=False, queue_num=q,
                    )

            call_i = [0]
            # batches are already in LPT (largest-first) order from _prep
            for bi, bj in enumerate(batches):
                b00, Bh0, b10, Bh1 = (int(v) for v in bat_g0[bi])
                xg = {}
                for h, g0, Bt in ((0, b00, Bh0), (1, b10, Bh1)):
                    if Bt == 0:
                        continue
                    t = xgpool.tile([P, Bmax, f], bf16, tag="xg",
                                    name=f"xg{bi}_{h}")
                    xg[h] = (t, g0)
                    # split each (batch, half) gather across SPLIT_CALLS
                    # calls on distinct queues for finer Q7-pair pipelining
                    if TGT_CALL > 0:
                        nsp = max(1, min(-(-Bt * P // TGT_CALL), Bt))
                    else:
                        nsp = max(1, min(SPLIT_CALLS, Bt))
                    bnds = [Bt * k // nsp for k in range(nsp + 1)]
                    for k in range(nsp):
                        lo, hi = bnds[k], bnds[k + 1]
                        if hi == lo:
                            continue
                        if exact:
                            ci = call_i[0]
                            call_i[0] += 1
                            # no min/max bounds: s_assert_within would emit
                            # runtime-assert ALU ops that halted the HW run
                            nreg = nc.gpsimd.value_load(
                                cnts_t[0:1, ci:ci + 1])
                        else:
                            nreg = (hi - lo) * P
                        ca, cb = (g0 + lo) * 8, (g0 + hi) * 8
                        if cb <= c0:
                            idx_ap = idx_a[:, ca:cb]
                        else:
                            idx_ap = idx_b[:, ca - c0:cb - c0]
                        nc.gpsimd.dma_gather(
                            t[:, lo:hi, :], x_lo if h == 0 else x_hi,
                            idx_ap,
                            (hi - lo) * P, nreg, f,
                            single_packet=False,
                            queue_num=next_q(hi - lo),
                        )
                for j in bj:
                    B0, B1 = int(cmax[j, 0]), int(cmax[j, 1])
                    ns_j = B0 + B1
                    if ns_j == 0:
                        ot = wout.tile([P, f], fp32, tag="ot", name=f"otz{j}")
                        nc.vector.tensor_copy(ot[:], biasb_t[:])
                        nc.sync.dma_start(out_d[j * P:(j + 1) * P, :], ot[:])
                        continue
                    s0 = int(slot0[j, 0])
                    oh = ohpool.tile([P, ns_j, P], bf16, tag="oh",
                                     name=f"oh{j}")
                    nc.sync.dma_start(
                        oh[:].rearrange("p q d -> p (q d)"),
                        oh_d[:, s0 * P:(s0 + ns_j) * P])
                    ps1 = ps1pool.tile([f, P], fp32, space="PSUM", tag="ps1",
                                       name=f"ps1_{j}")
                    k = 0
                    for h, B in ((0, B0), (1, B1)):
                        if B == 0:
                            continue
                        t, g0 = xg[h]
                        goff = int(gslot0[j, h]) - g0
                        for kk in range(B):
                            nc.tensor.matmul(
                                ps1[:],
                                lhsT=t[:, goff + kk, :],
                                rhs=oh[:, k, :],
                                start=(k == 0), stop=(k == ns_j - 1),
                            )
                            k += 1
                    hsT = wout.tile([f, P], bf16, tag="hsT", name=f"hsT{j}")
                    nc.scalar.copy(hsT[:], ps1[:])
                    ps2 = ps2pool.tile([P, f], fp32, space="PSUM", tag="ps2",
                                       name=f"ps2_{j}")
                    nc.tensor.matmul(ps2[:], lhsT=hsT[:], rhs=w_t[:],
                                     start=True, stop=True)
                    ot = wout.tile([P, f], fp32, tag="ot", name=f"ot{j}")
                    nc.vector.tensor_tensor(
                        out=ot[:], in0=ps2[:], in1=biasb_t[:],
                        op=mybir.AluOpType.add,
                    )
                    nc.sync.dma_start(out_d[j * P:(j + 1) * P, :], ot[:])
    nc.compile()
    return nc


# ------------------------------------------------------------------ kernel
def kernel(x, src, dst, weight, bias):
    _install_ntff_hook_shim()
    from concourse.bass_utils import run_bass_kernel_spmd

    x = np.asarray(x, np.float32)
    src = np.asarray(src, np.int32)
    dst = np.asarray(dst, np.int32)
    weight = np.asarray(weight, np.float32)
    bias = np.asarray(bias, np.float32)

    meta, in_maps = _prep(x, src, dst, weight, bias)
    key = (meta["n"], meta["f"], meta["e"],
           tuple(meta["cmax"].ravel().tolist()))
    if key not in _CACHE:
        _CACHE[key] = _build(meta)
    nc = _CACHE[key]

    trace = bool(int(os.environ.get("KERNEL_TRACE", "0")))
    res = run_bass_kernel_spmd(nc, in_maps, list(range(8)), trace=trace)
    global LAST_EXEC_NS, LAST_RESULTS
    LAST_EXEC_NS = res.exec_time_ns
    LAST_RESULTS = res

    n = meta["n"]
    wpc = meta["wpc"]
    f = meta["f"]
    pos_to_win = meta["pos_to_win"]
    out = np.zeros((meta["n_win"] * P, f), np.float32)
    for c in range(8):
        oc = res.results[c]["out"]
        for jj in range(wpc):
            w = pos_to_win[c, jj]
            if w >= 0:
                out[w * P:(w + 1) * P] = oc[jj * P:(jj + 1) * P]
    return np.ascontiguousarray(out[:n])


LAST_EXEC_NS = None
LAST_RESULTS = None


# revision 41
# speedup vs baseline: 1.0328x; 1.0328x over previous
"""DistGraphConv on 8 TRN2 NeuronCores.

GraphConv (norm='both'): out = rsqrt(deg_in) * ((A @ (x * rsqrt(deg_out))) @ W) + bias
                             = (A_sym @ x) @ W + bias,
where A_sym[d, s] = sum over edges (s->d) of rsqrt(deg_out[s]) * rsqrt(deg_in[d])
(the right norm commutes with the GEMM since it is a per-row scale).

Strategy (1-D dst partition, SPMD single NEFF on cores 0-7):
  - Nodes are split into 128-wide dst "windows"; window w -> (core, position)
    balanced by edge count; each core owns ~49 positions.
  - x is shipped as bf16 (representation change only).
  - Host prep (graph metadata only): bucket edges by (core, position, src-half),
    sort by src, build idx tables and the normalized adjacency one-hot blocks
    (values = rsqrt(deg_out[src])*rsqrt(deg_in[dst]), pure graph metadata).
    Chunk capacities are max-over-cores so all cores share one instruction
    stream; per-core differences are data only.
  - Device, per position:
      dma_gather     : Xg[e,:] = x_bf16[src_e,:]   (256B rows, HBM->SBUF)
      DMA            : O normalized one-hot stream (bf16)
      PE             : psum1[f,d] += Xg_chunk.T @ O_chunk   (h^T scatter-add)
      ACT            : hsT[f,d] = bf16(psum1)
      PE             : psum2[d,fo] = hsT.T @ W
      DVE            : ot = psum2 + bias (broadcast tile);  DMA out.

Performance notes (measured on trn2 via perfetto):
  - The kernel is bound by SWDGE descriptor emission for the per-edge
    gather: each dma_gather occupies one Q7 core pair (pair = queue_num),
    ~10.5 ns/index; 4 queues run 4 pairs in parallel -> ~100k idx/core
    / 4 pairs ~= 270 us floor.  Gather calls are split in half
    (SPLIT_CALLS=2 -> ~196 calls of ~550 idx) and issued LPT
    (largest-first) so the 4 queue makespans stay balanced and the
    drain tail hangs off the smallest positions.
  - Both GCN norms are folded into the one-hot values on the host, so
    the device path is gather -> matmul -> matmul -> +bias with no
    per-edge vector work.
  - Exact-count mode: idx tables pad with -1 and each call's true
    per-core count is passed via num_idxs_reg (value_load from SBUF,
    no min/max bounds -- bounded value_load emits runtime-assert ALU
    ops that halt the device).  The Q7 trims trailing -1s, skipping
    whole padded chunks (~4% emission + ~6% gather bytes).
  - Generation-1 xg tiles (first NXG tile allocations) pad with idx 0
    and a full count so every lane is written: no pool-priming
    memsets, and later generations can trim onto finite stale data.
"""

import os
import sys
import types

import numpy as np

P = 128
HALF = 32768  # int16 gather-index limit
NXG = int(os.environ.get("KERNEL_NXG", "16"))  # xg pool depth (buffers)
WARMUP = bool(int(os.environ.get("KERNEL_WARMUP", "0")))  # per-queue IRAM warmup
NEG_PAD = bool(int(os.environ.get("KERNEL_NEG_PAD", "1")))
EXACT_CNT = bool(int(os.environ.get("KERNEL_EXACT_CNT", "1")))
SPLIT_CALLS = int(os.environ.get("KERNEL_SPLIT_CALLS", "2"))  # gathers per (pos,half)
TGT_CALL = int(os.environ.get("KERNEL_TGT_CALL", "0"))  # target idxs/call (0=off)
QMODE = os.environ.get("KERNEL_QMODE", "load")  # load | rr

_CACHE: dict = {}


# ----------------------------------------------------------------- ntff shim
def _install_ntff_hook_shim():
    """The agent image's antenv lacks axon_hooks; bass_utils imports it when
    trace=True. Provide the module and register the ctypes NTFF hook."""
    try:
        from antenv.axon_hooks import get_axon_ntff_profile_hook  # noqa: F401
        return
    except ImportError:
        pass
    mod = types.ModuleType("antenv.axon_hooks")
    _hook = [None]
    mod.set_axon_ntff_profile_hook = lambda h: _hook.__setitem__(0, h)
    mod.get_axon_ntff_profile_hook = lambda: _hook[0]
    sys.modules["antenv.axon_hooks"] = mod
    import antenv

    antenv.axon_hooks = mod
    try:
        from trn_agent_boot.trn_boot import _ntff_profile_via_ctypes

        mod.set_axon_ntff_profile_hook(
            _ntff_profile_via_ctypes("/opt/axon/libaxon_pjrt.so")
        )
    except Exception:
        pass


# ----------------------------------------------------------------- host prep
def _prep(x, src, dst, weight, bias):
    import ml_dtypes

    n, f = x.shape
    e = src.shape[0]
    n_win = -(-n // P)
    cores = 8
    wpc = -(-n_win // cores)

    deg_out = np.maximum(np.bincount(src, minlength=n), 1).astype(np.float32)
    deg_in = np.maximum(np.bincount(dst, minlength=n), 1).astype(np.float32)
    w_edge = 1.0 / np.sqrt(deg_out[src] * deg_in[dst])  # normalized A values

    win = (dst >> 7).astype(np.int64)
    wcnt = np.bincount(win, minlength=n_win)

    # window -> (core, pos): sort windows by size desc; group of 8 similar
    # windows per position; within a group assign biggest to least-loaded core.
    worder = np.argsort(-wcnt, kind="stable")
    win_to_core = np.zeros(n_win, np.int64)
    win_to_pos = np.zeros(n_win, np.int64)
    pos_to_win = np.full((cores, wpc), -1, np.int64)
    core_load = np.zeros(cores, np.int64)
    for j in range(wpc):
        grp = worder[j * 8:(j + 1) * 8]
        order_c = np.argsort(core_load, kind="stable")
        for i, w in enumerate(grp):
            c = int(order_c[i])
            win_to_core[w] = c
            win_to_pos[w] = j
            pos_to_win[c, j] = w
            core_load[c] += wcnt[w]

    core = win_to_core[win]
    pos = win_to_pos[win]
    half = (src >= HALF).astype(np.int64)

    gkey = (core * wpc + pos) * 2 + half
    order = np.lexsort((src, gkey))
    src_s = src[order]
    dst_s = dst[order]
    w_s = w_edge[order]
    gkey_s = gkey[order]

    n_groups = cores * wpc * 2
    gcnt = np.bincount(gkey_s, minlength=n_groups)
    gstart = np.zeros(n_groups + 1, np.int64)
    np.cumsum(gcnt, out=gstart[1:])
    cnt = gcnt.reshape(cores, wpc, 2)
    cmax = (-(-cnt // P)).max(axis=0)  # [wpc, 2] chunks per (pos, half)
    slot0 = np.zeros((wpc, 2), np.int64)
    s = 0
    for j in range(wpc):
        for h in range(2):
            slot0[j, h] = s
            s += cmax[j, h]
    n_slots = int(s)

    # per-edge placement
    g_c = gkey_s // (wpc * 2)
    g_rem = gkey_s - g_c * (wpc * 2)
    g_j = g_rem >> 1
    g_h = g_rem & 1
    epos = np.arange(e, dtype=np.int64) - gstart[gkey_s]
    lane = epos & 127
    slot = slot0[g_j, g_h] + (epos >> 7)

    # dense normalized one-hot blocks: O[lane, slot*128+dstl] = w_edge
    o_rep = np.zeros((cores, P, n_slots * P), ml_dtypes.bfloat16)
    o_rep[g_c, lane, slot * P + (dst_s & 127)] = w_s

    # gather batching + call splitting. gather-slot order: per batch:
    # [h0: pos j0..][h1: pos j0..]; onehot slot order stays (j, h)-global.
    BPG = int(os.environ.get("KERNEL_BPG", "1"))
    batches = [list(range(b, min(b + BPG, wpc))) for b in range(0, wpc, BPG)]
    # LPT dispatch order baked into the layout: biggest batch first, so the
    # gather-slot/idx-table order equals issue order and the first batch's
    # idx columns sit at the front (loaded as a separate small tile).
    batches.sort(key=lambda bj: -sum(int(cmax[j, 0] + cmax[j, 1])
                                     for j in bj))
    # ...except the very first dispatched call executes synchronously on
    # the pool engine (head-of-line): lead with the SMALLEST batch so that
    # serial prefix is as short as possible, then largest-first.
    batches = batches[-1:] + batches[:-1]
    gslot0 = np.zeros((wpc, 2), np.int64)
    bat_g0 = []  # per batch: (g0_h0, B_h0, g0_h1, B_h1)
    s = 0
    for bj in batches:
        b00 = s
        for j in bj:
            gslot0[j, 0] = s
            s += cmax[j, 0]
        b10 = s
        for j in bj:
            gslot0[j, 1] = s
            s += cmax[j, 1]
        bat_g0.append((b00, b10 - b00, b10, s - b10))
    assert s == n_slots

    # per-call real index counts (exact-count mode: num_idxs_reg from SBUF).
    # Call order mirrors the device loop: per batch, per half, per split.
    # Exact counts require trailing-only padding within each call: BPG == 1.
    # The first NXG (batch, half) tiles are generation 1 of the xg pool:
    # they pad with idx 0 and a FULL count so every lane gets written,
    # which removes the need to prime the pool with memsets (later
    # generations overwrite finite gather data, so trimming is safe).
    exact = EXACT_CNT and NEG_PAD and BPG == 1
    call_list = []  # (j, h, lo, hi, full) chunk ranges
    zero_pad = {}  # (j, h) -> True for generation-1 tiles
    if exact:
        tile_ord = 0
        for bj in batches:
            (j,) = bj
            for h in range(2):
                Bt = int(cmax[j, h])
                if Bt == 0:
                    continue
                tile_ord += 1
                full = tile_ord <= NXG
                zero_pad[(j, h)] = full
                nsp = max(1, min(SPLIT_CALLS, Bt))
                bnds = [Bt * k // nsp for k in range(nsp + 1)]
                for k in range(nsp):
                    lo, hi = bnds[k], bnds[k + 1]
                    if hi > lo:
                        call_list.append((j, h, lo, hi, full))
    n_calls = max(1, len(call_list))
    cnts = np.zeros((cores, 1, n_calls), np.int32)
    for ci, (j, h, lo, hi, full) in enumerate(call_list):
        if full:
            cnts[:, 0, ci] = (hi - lo) * P
        else:
            r = cnt[:, j, h] - lo * P
            cnts[:, 0, ci] = np.clip(r, 0, (hi - lo) * P)

    # idx tables in gather-slot order [cores, 16, idx_cols], pad -1 (trimmed)
    pad_val = -1 if NEG_PAD else 0
    idx_cols = n_slots * 8
    idx_tab = np.full((cores, 16, idx_cols), pad_val, np.int16)
    for c in range(cores):
        for j in range(wpc):
            for h in range(2):
                B = int(cmax[j, h])
                if B == 0:
                    continue
                g = (c * wpc + j) * 2 + h
                i0, i1 = gstart[g], gstart[g + 1]
                pv = 0 if zero_pad.get((j, h)) else pad_val
                buf = np.full(B * P, pv, np.int16)
                buf[: i1 - i0] = (src_s[i0:i1] - h * HALF).astype(np.int16)
                cc = int(gslot0[j, h]) * 8
                idx_tab[c, :, cc:cc + B * 8] = buf.reshape(B * 8, 16).T
    idx_tab_full = np.tile(idx_tab, (1, 8, 1))

    bias_b = np.tile(np.asarray(bias, np.float32)[None, :], (P, 1))
    w_bf = np.asarray(weight, np.float32).astype(ml_dtypes.bfloat16)
    x_bf = np.asarray(x, np.float32).astype(ml_dtypes.bfloat16)

    meta = dict(
        n=n, f=f, e=e, n_win=n_win, wpc=wpc, n_slots=n_slots,
        idx_cols=idx_cols, cmax=cmax, slot0=slot0, gslot0=gslot0,
        batches=batches, bat_g0=bat_g0, pos_to_win=pos_to_win,
        exact=exact, n_calls=n_calls,
    )
    in_maps = []
    for c in range(cores):
        m = {
            "x": x_bf,
            "onehot": o_rep[c],
            "idx": idx_tab_full[c],
            "w_bf": w_bf,
            "bias_b": bias_b,
        }
        if exact:
            m["cnts"] = cnts[c]
        in_maps.append(m)
    return meta, in_maps


# ------------------------------------------------------------- device build
def _build(meta):
    import concourse.bacc as bacc
    import concourse.mybir as mybir
    import concourse.tile as tile
    from concourse.library_config import mlp

    n, f = meta["n"], meta["f"]
    wpc = meta["wpc"]
    n_slots = meta["n_slots"]
    idx_cols = meta["idx_cols"]
    cmax = meta["cmax"]
    slot0 = meta["slot0"]
    gslot0 = meta["gslot0"]
    batches = meta["batches"]
    bat_g0 = meta["bat_g0"]
    fp32 = mybir.dt.float32
    bf16 = mybir.dt.bfloat16

    exact = meta["exact"]
    n_calls = meta["n_calls"]

    nc = bacc.Bacc("TRN2", target_bir_lowering=False, debug=False,
                   num_swdge_queues=4)
    x_d = nc.declare_dram_parameter("x", [n, f], bf16, isOutput=False)
    cnts_d = (nc.declare_dram_parameter("cnts", [1, n_calls], mybir.dt.int32,
                                        isOutput=False) if exact else None)
    oh_d = nc.declare_dram_parameter("onehot", [P, n_slots * P], bf16,
                                     isOutput=False)
    idx_d = nc.declare_dram_parameter("idx", [P, idx_cols], mybir.dt.int16,
                                      isOutput=False)
    w_d = nc.declare_dram_parameter("w_bf", [f, f], bf16, isOutput=False)
    biasb_d = nc.declare_dram_parameter("bias_b", [P, f], fp32, isOutput=False)
    out_d = nc.declare_dram_parameter("out", [wpc * P, f], fp32, isOutput=True)

    x_lo = x_d[0:min(HALF, n), :]
    x_hi = x_d[HALF:n, :] if n > HALF else None

    Bmax = max(max(bg[1], bg[3]) for bg in bat_g0)  # chunks per gather call
    gq = [0, 0, 0, 0]
    rr = [0]

    def next_q(nidx):
        if QMODE == "rr":
            q = rr[0] % 4
            rr[0] += 1
        else:
            q = min(range(4), key=lambda i: gq[i])
        gq[q] += nidx
        return q

    with tile.TileContext(nc) as tc:
        nc.gpsimd.load_library(mlp)
        with (
            tc.tile_pool(name="const", bufs=1) as cpool,
            tc.tile_pool(name="xg", bufs=NXG) as xgpool,
            tc.tile_pool(name="oh", bufs=6) as ohpool,
            tc.tile_pool(name="wout", bufs=6) as wout,
            tc.tile_pool(name="ps1", bufs=6, space="PSUM") as ps1pool,
            tc.tile_pool(name="ps2", bufs=2, space="PSUM") as ps2pool,
        ):
            # one-time loads; counts (tiny, gates every gather) first.
            # idx split into two TILES: Tile deps are tile-granular, so the
            # first batch's gathers only wait on the small idx_a DMA.
            cnts_t = None
            if exact:
                cnts_t = cpool.tile([1, n_calls], mybir.dt.int32)
                nc.sync.dma_start(cnts_t[:], cnts_d[:])
            c0 = int(bat_g0[0][3] + bat_g0[0][2] - bat_g0[0][0]) * 8
            c0 = max(8, min(c0, idx_cols))
            idx_a = cpool.tile([P, c0], mybir.dt.int16)
            nc.sync.dma_start(idx_a[:], idx_d[:, 0:c0])
            idx_b = None
            if idx_cols > c0:
                idx_b = cpool.tile([P, idx_cols - c0], mybir.dt.int16)
                nc.sync.dma_start(idx_b[:], idx_d[:, c0:])
            w_t = cpool.tile([f, f], bf16)
            nc.sync.dma_start(w_t[:], w_d[:])
            biasb_t = cpool.tile([P, f], fp32)
            nc.sync.dma_start(biasb_t[:], biasb_d[:])

            if NEG_PAD and not exact:
                # prime the xg pool buffers so skipped lanes stay finite
                # (exact mode instead writes every lane of generation-1
                # tiles via zero-pad + full counts -- no priming needed)
                for i in range(NXG):
                    t = xgpool.tile([P, Bmax, f], bf16, tag="xg",
                                    name=f"xgz{i}")
                    nc.vector.memset(t[:], 0.0)

            if WARMUP:
                # one tiny gather per queue: pays each Q7 pair's ~6us
                # library-IRAM load while the idx table is still streaming in
                for q in range(4):
                    wut = cpool.tile([P, 1, f], bf16)
                    nc.gpsimd.dma_gather(
                        wut[:, 0:1, :], x_lo,
                        idx_a[:, 0:8],
                        P, P, f, single_packet=False, queue_num=q,
                    )

            call_i = [0]
            # batches are already in LPT (largest-first) order from _prep
            for bi, bj in enumerate(batches):
                b00, Bh0, b10, Bh1 = (int(v) for v in bat_g0[bi])
                xg = {}
                for h, g0, Bt in ((0, b00, Bh0), (1, b10, Bh1)):
                    if Bt == 0:
                        continue
                    t = xgpool.tile([P, Bmax, f], bf16, tag="xg",
                                    name=f"xg{bi}_{h}")
                    xg[h] = (t, g0)
                    # split each (batch, half) gather across SPLIT_CALLS
                    # calls on distinct queues for finer Q7-pair pipelining
                    if TGT_CALL > 0:
                        nsp = max(1, min(-(-Bt * P // TGT_CALL), Bt))
                    else:
                        nsp = max(1, min(SPLIT_CALLS, Bt))
                    bnds = [Bt * k // nsp for k in range(nsp + 1)]
                    for k in range(nsp):
                        lo, hi = bnds[k], bnds[k + 1]
                        if hi == lo:
                            continue
                        if exact:
                            ci = call_i[0]
                            call_i[0] += 1
                            # no min/max bounds: s_assert_within would emit
                            # runtime-assert ALU ops that halted the HW run
                            nreg = nc.gpsimd.value_load(
                                cnts_t[0:1, ci:ci + 1])
                        else:
                            nreg = (hi - lo) * P
                        ca, cb = (g0 + lo) * 8, (g0 + hi) * 8
                        if cb <= c0:
                            idx_ap = idx_a[:, ca:cb]
                        else:
                            idx_ap = idx_b[:, ca - c0:cb - c0]
                        nc.gpsimd.dma_gather(
                            t[:, lo:hi, :], x_lo if h == 0 else x_hi,
                            idx_ap,
                            (hi - lo) * P, nreg, f,
                            single_packet=False,
                            queue_num=next_q(hi - lo),
                        )
                for j in bj:
                    B0, B1 = int(cmax[j, 0]), int(cmax[j, 1])
                    ns_j = B0 + B1
                    if ns_j == 0:
                        ot = wout.tile([P, f], fp32, tag="ot", name=f"otz{j}")
                        nc.vector.tensor_copy(ot[:], biasb_t[:])
                        nc.sync.dma_start(out_d[j * P:(j + 1) * P, :], ot[:])
                        continue
                    s0 = int(slot0[j, 0])
                    oh = ohpool.tile([P, ns_j, P], bf16, tag="oh",
                                     name=f"oh{j}")
                    nc.sync.dma_start(
                        oh[:].rearrange("p q d -> p (q d)"),
                        oh_d[:, s0 * P:(s0 + ns_j) * P])
                    ps1 = ps1pool.tile([f, P], fp32, space="PSUM", tag="ps1",
                                       name=f"ps1_{j}")
                    k = 0
                    for h, B in ((0, B0), (1, B1)):
                        if B == 0:
                            continue
                        t, g0 = xg[h]
                        goff = int(gslot0[j, h]) - g0
                        for kk in range(B):
                            nc.tensor.matmul(
                                ps1[:],
                                lhsT=t[:, goff + kk, :],
                                rhs=oh[:, k, :],
                                start=(k == 0), stop=(k == ns_j - 1),
                            )
                            k += 1
                    hsT = wout.tile([f, P], bf16, tag="hsT", name=f"hsT{j}")
                    nc.scalar.copy(hsT[:], ps1[:])
                    ps2 = ps2pool.tile([P, f], fp32, space="PSUM", tag="ps2",
                                       name=f"ps2_{j}")
                    nc.tensor.matmul(ps2[:], lhsT=hsT[:], rhs=w_t[:],
                                     start=True, stop=True)
                    ot = wout.tile([P, f], fp32, tag="ot", name=f"ot{j}")
                    nc.vector.tensor_tensor(
                        out=ot[:], in0=ps2[:], in1=biasb_t[:],
                        op=mybir.AluOpType.add,
                    )
                    nc.sync.dma_start(out_d[j * P:(j + 1) * P, :], ot[:])
    nc.compile()
    return nc


# ------------------------------------------------------------------ kernel
def kernel(x, src, dst, weight, bias):
    _install_ntff_hook_shim()
    from concourse.bass_utils import run_bass_kernel_spmd

    x = np.asarray(x, np.float32)
    src = np.asarray(src, np.int32)
    dst = np.asarray(dst, np.int32)
    weight = np.asarray(weight, np.float32)
    bias = np.asarray(bias, np.float32)

    meta, in_maps = _prep(x, src, dst, weight, bias)
    key = (meta["n"], meta["f"], meta["e"],
           tuple(meta["cmax"].ravel().tolist()))
    if key not in _CACHE:
        _CACHE[key] = _build(meta)
    nc = _CACHE[key]

    trace = bool(int(os.environ.get("KERNEL_TRACE", "0")))
    res = run_bass_kernel_spmd(nc, in_maps, list(range(8)), trace=trace)
    global LAST_EXEC_NS, LAST_RESULTS
    LAST_EXEC_NS = res.exec_time_ns
    LAST_RESULTS = res

    n = meta["n"]
    wpc = meta["wpc"]
    f = meta["f"]
    pos_to_win = meta["pos_to_win"]
    out = np.zeros((meta["n_win"] * P, f), np.float32)
    for c in range(8):
        oc = res.results[c]["out"]
        for jj in range(wpc):
            w = pos_to_win[c, jj]
            if w >= 0:
                out[w * P:(w + 1) * P] = oc[jj * P:(jj + 1) * P]
    return np.ascontiguousarray(out[:n])


LAST_EXEC_NS = None
LAST_RESULTS = None
